# revision 42
# baseline (speedup 1.0000x reference)
"""Trainium2 Bass kernel for nn_CausalSE (chunked-EMA squeeze-excite gating).

Reference computation (per batch b):
    xc   = mean over chunks of 16 along L            -> [C, N]   (N = L/16)
    e_t  = g*e_{t-1} + (1-g)*xc_t   (causal EMA)     -> [C, N]
    h    = relu(w1 @ e + b1)                         -> [C/8, N]
    gate = sigmoid(w2 @ h + b2)                      -> [C, N]
    out  = repeat(gate, 16) * x                      -> [C, L]

Distribution: pure data-parallel over batch. B == 8 == n_cores, each core
processes one full batch element independently; no collectives.

Math transform used on-chip: let u_t = g*u_{t-1} + sum16(x)_t (plain scan on
pooled *sums*).  Then e = ((1-g)/16) * u, which is folded into w1 on the host
(w1s = w1 * ((1-g)/16)).  This removes a per-element rescale pass on DVE.
"""

import numpy as np
from contextlib import ExitStack

import concourse.bass as bass
import concourse.tile as tile
from concourse import bacc, mybir

F32 = mybir.dt.float32
P = 128


def build_graph(C=512, L=8192, CS=16, HID=64, NL=4, reps=1, chunks=None,
                se_bf16=True, out_eng="scalar", in_eng="sync", xbufs=2, sbufs=2, pbufs=2,
                resident=False, hoist_in=False, dma_only=False,
                gp_gate=0, serialize=False):
    """Build the per-core Bass graph (SPMD: every core runs this same graph).

    NL: number of equal column chunks (ignored if `chunks` given).
    chunks: explicit list of column widths (each a multiple of CS, sum == L).
    reps: repeat the whole computation (for on-device timing via slope).
    se_bf16: run the SE matmuls in bf16 (scan output downcasts for free).
    """
    NCT = C // P          # channel partition-tiles
    if chunks is None:
        chunks = [L // NL] * NL
    assert sum(chunks) == L and all(c % CS == 0 for c in chunks)
    NL = len(chunks)
    LCmax = max(chunks)
    NCmax = LCmax // CS
    MMDT = mybir.dt.bfloat16 if se_bf16 else F32

    nc = bacc.Bacc(None, target_bir_lowering=False)

    x_ext = nc.declare_dram_parameter("x", [C, L], F32, isOutput=False)
    w1_ext = nc.declare_dram_parameter("w1s", [P, NCT * HID], MMDT, isOutput=False)
    w2_ext = nc.declare_dram_parameter("w2t", [HID, C], MMDT, isOutput=False)
    b1_ext = nc.declare_dram_parameter("b1", [HID, 1], F32, isOutput=False)
    b2_ext = nc.declare_dram_parameter("b2", [P, NCT], F32, isOutput=False)
    g_ext = nc.declare_dram_parameter("g", [P, NCT], F32, isOutput=False)
    out_ext = nc.declare_dram_parameter("out", [C, L], F32, isOutput=True)

    _engs = {
        "scalar": [nc.scalar],
        "sync": [nc.sync],
        "alt": [nc.scalar, nc.sync],
        "gpsimd": [nc.gpsimd],
        "alt3": [nc.scalar, nc.sync, nc.gpsimd],
        "sg": [nc.sync, nc.gpsimd],
        "cg": [nc.scalar, nc.gpsimd],
    }
    out_engines = _engs[out_eng]
    in_engines = _engs[in_eng]
    oe_idx = 0
    ie_idx = 0

    with ExitStack() as ctx:
        tc = ctx.enter_context(tile.TileContext(nc))
        consts = ctx.enter_context(tc.tile_pool(name="consts", bufs=1))
        xpool = ctx.enter_context(tc.tile_pool(name="xpool", bufs=xbufs))
        small = ctx.enter_context(tc.tile_pool(name="small", bufs=sbufs))
        psum = ctx.enter_context(
            tc.tile_pool(name="psum", bufs=pbufs, space=bass.MemorySpace.PSUM)
        )

        w1_sb = consts.tile([P, NCT, HID], MMDT)
        nc.gpsimd.dma_start(
            out=w1_sb[:], in_=w1_ext[:].rearrange("p (ct h) -> p ct h", ct=NCT)
        )
        w2_sb = consts.tile([HID, C], MMDT)
        nc.gpsimd.dma_start(out=w2_sb[:], in_=w2_ext[:])
        b1_sb = consts.tile([HID, 1], F32)
        nc.gpsimd.dma_start(out=b1_sb[:], in_=b1_ext[:])
        b2_sb = consts.tile([P, NCT], F32)
        nc.gpsimd.dma_start(out=b2_sb[:], in_=b2_ext[:])
        g_sb = consts.tile([P, NCT], F32)
        nc.gpsimd.dma_start(out=g_sb[:], in_=g_ext[:])

        # broadcast gamma along the free axis for the scan's data0 operand
        ones = consts.tile([P, NCmax], F32)
        nc.vector.memset(ones[:], 1.0)
        g_bcast = []
        for ct in range(NCT):
            gb = consts.tile([P, NCmax], F32, tag=f"gb{ct}")
            nc.vector.tensor_scalar_mul(gb[:], ones[:], g_sb[:, ct : ct + 1])
            g_bcast.append(gb)

        from concourse.tile_rust import add_dep_helper

        prev_rep_last_out = None
        for _r in range(reps):
            rep_in_insts = []
            last_out_inst = None
            u_prev = [None] * NCT
            prev_nc = 0
            col = 0
            x_big = None
            if resident:
                x_big = [
                    xpool.tile([P, L], F32, tag=f"xbig{ct}", name=f"xbig{ct}")
                    for ct in range(NCT)
                ]
            if hoist_in:
                assert resident
                c0 = 0
                for LC in chunks:
                    for ct in range(NCT):
                        in_engines[ie_idx % len(in_engines)].dma_start(
                            out=x_big[ct][:, c0 : c0 + LC],
                            in_=x_ext[ct * P : (ct + 1) * P, c0 : c0 + LC],
                        )
                        ie_idx += 1
                    c0 += LC
            for k, LC in enumerate(chunks):
                NCc = LC // CS
                xts = []
                uts = []
                for ct in range(NCT):
                    if resident:
                        x_t = x_big[ct][:, col : col + LC]
                    else:
                        x_t = xpool.tile([P, LC], F32, tag=f"x{ct}", name=f"x{ct}")[:]
                    if not hoist_in:
                        in_inst = in_engines[ie_idx % len(in_engines)].dma_start(
                            out=x_t,
                            in_=x_ext[ct * P : (ct + 1) * P, col : col + LC],
                        )
                        ie_idx += 1
                        if serialize and prev_rep_last_out is not None:
                            add_dep_helper(
                                in_inst.ins,
                                prev_rep_last_out.ins,
                                reason="serialize reps for single-shot timing",
                            )
                    if dma_only:
                        out_engines[oe_idx % len(out_engines)].dma_start(
                            out=out_ext[ct * P : (ct + 1) * P, col : col + LC],
                            in_=x_t,
                        )
                        oe_idx += 1
                        xts.append(x_t)
                        continue
                    xc_t = small.tile([P, NCc], F32, tag=f"xc{ct}")
                    nc.vector.tensor_reduce(
                        out=xc_t[:],
                        in_=x_t.rearrange("p (n j) -> p n j", j=CS),
                        axis=mybir.AxisListType.X,
                        op=mybir.AluOpType.add,
                    )
                    u_t = small.tile([P, NCc], MMDT, tag=f"u{ct}")
                    init = 0.0 if k == 0 else u_prev[ct][:, prev_nc - 1 : prev_nc]
                    nc.vector.tensor_tensor_scan(
                        out=u_t[:],
                        data0=g_bcast[ct][:, :NCc],
                        data1=xc_t[:],
                        initial=init,
                        op0=mybir.AluOpType.mult,
                        op1=mybir.AluOpType.add,
                    )
                    xts.append(x_t)
                    uts.append(u_t)
                if dma_only:
                    col += LC
                    continue

                # SE bottleneck: h = relu(w1s @ u + b1)
                h_ps = psum.tile([HID, NCc], F32, tag="hps")
                for ct in range(NCT):
                    nc.tensor.matmul(
                        h_ps[:],
                        w1_sb[:, ct, :],
                        uts[ct][:],
                        start=(ct == 0),
                        stop=(ct == NCT - 1),
                    )
                h_sb = small.tile([HID, NCc], MMDT, tag="h")
                nc.scalar.activation(
                    out=h_sb[:],
                    in_=h_ps[:],
                    func=mybir.ActivationFunctionType.Relu,
                    bias=b1_sb[:],
                )
                for ct in range(NCT):
                    o_ps = psum.tile([P, NCc], F32, tag="ops")
                    nc.tensor.matmul(
                        o_ps[:],
                        w2_sb[:, ct * P : (ct + 1) * P],
                        h_sb[:],
                        start=True,
                        stop=True,
                    )
                    gate_t = small.tile([P, NCc], F32, tag="gate")
                    nc.scalar.activation(
                        out=gate_t[:],
                        in_=o_ps[:],
                        func=mybir.ActivationFunctionType.Sigmoid,
                        bias=b2_sb[:, ct : ct + 1],
                    )
                    x3 = xts[ct].rearrange("p (n j) -> p n j", j=CS)
                    g_ap = gate_t[:]
                    gate_b = bass.AP(
                        tensor=g_ap.tensor,
                        offset=g_ap.offset,
                        ap=[list(g_ap.ap[0]), list(g_ap.ap[1]), [0, CS]],
                    )
                    gate_eng = nc.gpsimd if ct < gp_gate else nc.vector
                    gate_eng.tensor_tensor(
                        out=x3, in0=x3, in1=gate_b, op=mybir.AluOpType.mult
                    )
                    last_out_inst = out_engines[oe_idx % len(out_engines)].dma_start(
                        out=out_ext[ct * P : (ct + 1) * P, col : col + LC],
                        in_=xts[ct],
                    )
                    oe_idx += 1
                u_prev = uts
                prev_nc = NCc
                col += LC
            prev_rep_last_out = last_out_inst

    nc.compile()
    return nc


def host_prep(gamma, w1, b1, w2, b2, C=512, HID=64, se_bf16=True):
    """Host-side preprocessing of the shared (small) tensors."""
    import ml_dtypes

    NCT = C // P
    mmdt = ml_dtypes.bfloat16 if se_bf16 else np.float32
    gamma = np.asarray(gamma, np.float32)
    w1 = np.asarray(w1, np.float32)
    w2 = np.asarray(w2, np.float32)
    bv = (1.0 - gamma) / 16.0
    w1s = (w1 * bv[None, :]).T  # [C, HID]
    # [C, HID] -> [P, NCT*HID] with c = ct*P + p
    w1s_r = np.ascontiguousarray(
        w1s.reshape(NCT, P, HID).transpose(1, 0, 2).reshape(P, NCT * HID)
    ).astype(mmdt)
    w2t = np.ascontiguousarray(w2.T).astype(mmdt)  # [HID, C]
    b1_r = np.ascontiguousarray(np.asarray(b1, np.float32).reshape(HID, 1))
    b2_r = np.ascontiguousarray(np.asarray(b2, np.float32).reshape(NCT, P).T)
    g_r = np.ascontiguousarray(gamma.reshape(NCT, P).T)
    return w1s_r, w2t, b1_r, b2_r, g_r


DEFAULT_NL = 4
DEFAULT_CFG = dict(
    chunks=[512, 1024, 1536, 2048, 1536, 1024, 512],
    in_eng="alt",
    out_eng="alt",
    xbufs=1,
    sbufs=3,
    pbufs=3,
    se_bf16=False,
    resident=True,
)

_GRAPH_CACHE = {}


def _get_graph(reps=1):
    key = reps
    if key not in _GRAPH_CACHE:
        _GRAPH_CACHE[key] = build_graph(reps=reps, **DEFAULT_CFG)
    return _GRAPH_CACHE[key]


def make_in_maps(x, gamma, w1, b1, w2, b2):
    B, C, L = x.shape
    HID = w1.shape[0]
    w1s_r, w2t, b1_r, b2_r, g_r = host_prep(
        gamma, w1, b1, w2, b2, C=C, HID=HID, se_bf16=DEFAULT_CFG["se_bf16"]
    )
    x = np.asarray(x, np.float32)
    return [
        {
            "x": np.ascontiguousarray(x[b]),
            "w1s": w1s_r,
            "w2t": w2t,
            "b1": b1_r,
            "b2": b2_r,
            "g": g_r,
        }
        for b in range(B)
    ]


_RUNNER_CACHE = {}


def _make_runner(nc, n_cores):
    """Persistent jitted SPMD runner for `nc` across `n_cores` devices.

    Returns run(in_maps) -> list[dict] of per-core outputs.
    """
    import jax
    from jax.sharding import Mesh, PartitionSpec
    from jax.experimental.shard_map import shard_map
    from concourse import bass2jax

    bass2jax.install_neuronx_cc_hook()

    partition_name = nc.partition_id_tensor.name if nc.partition_id_tensor else None
    in_names, out_names, out_avals = [], [], []
    for alloc in nc.m.functions[0].allocations:
        if not isinstance(alloc, mybir.MemoryLocationSet):
            continue
        name = alloc.memorylocations[0].name
        if alloc.kind == "ExternalInput":
            if name != partition_name:
                in_names.append(name)
        elif alloc.kind == "ExternalOutput":
            out_names.append(name)
            out_avals.append(
                jax.core.ShapedArray(tuple(alloc.tensor_shape), mybir.dt.np(alloc.dtype))
            )
    n_params = len(in_names)
    in_names_all = in_names + out_names
    if partition_name is not None:
        in_names_all.append(partition_name)

    def _body(*args):
        operands = list(args)
        if partition_name is not None:
            operands.append(bass2jax.partition_id_tensor())
        outs = bass2jax._bass_exec_p.bind(
            *operands,
            out_avals=tuple(out_avals),
            in_names=tuple(in_names_all),
            out_names=tuple(out_names),
            lowering_input_output_aliases=(),
            sim_require_finite=True,
            sim_require_nnan=True,
            nc=nc,
        )
        return tuple(outs)

    devices = jax.devices()[:n_cores]
    mesh = Mesh(np.asarray(devices), ("core",))
    n_outs = len(out_avals)
    sharded = jax.jit(
        shard_map(
            _body,
            mesh=mesh,
            in_specs=(PartitionSpec("core"),) * (n_params + n_outs),
            out_specs=(PartitionSpec("core"),) * len(out_names),
            check_rep=False,
        ),
        keep_unused=True,
    )
    concat_zeros = [
        np.zeros((n_cores * a.shape[0], *a.shape[1:]), a.dtype) for a in out_avals
    ]

    def run(in_maps):
        per_core = [[np.asarray(m[name]) for name in in_names] for m in in_maps]
        concat_in = [
            np.concatenate([per_core[c][i] for c in range(n_cores)], axis=0)
            for i in range(n_params)
        ]
        out_arrs = sharded(*concat_in, *concat_zeros)
        return [
            {
                name: np.asarray(out_arrs[i]).reshape(
                    n_cores, *out_avals[i].shape
                )[c]
                for i, name in enumerate(out_names)
            }
            for c in range(n_cores)
        ]

    return run


def _get_runner(reps=1, n_cores=8):
    key = (reps, n_cores)
    if key not in _RUNNER_CACHE:
        _RUNNER_CACHE[key] = _make_runner(_get_graph(reps=reps), n_cores)
    return _RUNNER_CACHE[key]


def kernel(x, gamma, w1, b1, w2, b2):
    x = np.asarray(x)
    B, C, L = x.shape
    assert (B, C, L) == (8, 512, 8192), (B, C, L)
    in_maps = make_in_maps(x, gamma, w1, b1, w2, b2)
    try:
        res = _get_runner(reps=1, n_cores=B)(in_maps)
    except Exception:
        # fallback: the official (slower to dispatch, identical NEFF) path
        from concourse.bass_utils import run_bass_kernel_spmd

        res = run_bass_kernel_spmd(
            _get_graph(reps=1), in_maps, core_ids=list(range(B))
        ).results
    out = np.stack([res[b]["out"] for b in range(B)], axis=0)
    return np.ascontiguousarray(out, dtype=np.float32)


# revision 50
# speedup vs baseline: 1.0166x; 1.0166x over previous
"""Trainium2 Bass kernel for nn_CausalSE (chunked-EMA squeeze-excite gating).

Reference computation (per batch b):
    xc   = mean over chunks of 16 along L            -> [C, N]   (N = L/16)
    e_t  = g*e_{t-1} + (1-g)*xc_t   (causal EMA)     -> [C, N]
    h    = relu(w1 @ e + b1)                         -> [C/8, N]
    gate = sigmoid(w2 @ h + b2)                      -> [C, N]
    out  = repeat(gate, 16) * x                      -> [C, L]

Distribution: pure data-parallel over batch. B == 8 == n_cores, each core
processes one full batch element independently; no collectives.

Math transform used on-chip: let u_t = g*u_{t-1} + sum16(x)_t (plain scan on
pooled *sums*).  Then e = ((1-g)/16) * u, which is folded into w1 on the host
(w1s = w1 * ((1-g)/16)).  This removes a per-element rescale pass on DVE.
"""

import numpy as np
from contextlib import ExitStack

import concourse.bass as bass
import concourse.tile as tile
from concourse import bacc, mybir

F32 = mybir.dt.float32
P = 128


def build_graph(C=512, L=8192, CS=16, HID=64, NL=4, reps=1, chunks=None,
                se_bf16=True, out_eng="scalar", in_eng="sync", xbufs=2, sbufs=2, pbufs=2,
                resident=False, hoist_in=False, dma_only=False,
                gp_gate=0, serialize=False, g_direct=False, fast_head=False):
    """Build the per-core Bass graph (SPMD: every core runs this same graph).

    NL: number of equal column chunks (ignored if `chunks` given).
    chunks: explicit list of column widths (each a multiple of CS, sum == L).
    reps: repeat the whole computation (for on-device timing via slope).
    se_bf16: run the SE matmuls in bf16 (scan output downcasts for free).
    """
    NCT = C // P          # channel partition-tiles
    if chunks is None:
        chunks = [L // NL] * NL
    assert sum(chunks) == L and all(c % CS == 0 for c in chunks)
    NL = len(chunks)
    LCmax = max(chunks)
    NCmax = LCmax // CS
    MMDT = mybir.dt.bfloat16 if se_bf16 else F32

    nc = bacc.Bacc(None, target_bir_lowering=False)

    x_ext = nc.declare_dram_parameter("x", [C, L], F32, isOutput=False)
    w1_ext = nc.declare_dram_parameter("w1s", [P, NCT * HID], MMDT, isOutput=False)
    w2_ext = nc.declare_dram_parameter("w2t", [HID, C], MMDT, isOutput=False)
    b1_ext = nc.declare_dram_parameter("b1", [HID, 1], F32, isOutput=False)
    b2_ext = nc.declare_dram_parameter("b2", [P, NCT], F32, isOutput=False)
    g_ext = nc.declare_dram_parameter("g", [P, NCT], F32, isOutput=False)
    out_ext = nc.declare_dram_parameter("out", [C, L], F32, isOutput=True)

    _engs = {
        "scalar": [nc.scalar],
        "sync": [nc.sync],
        "alt": [nc.scalar, nc.sync],
        "gpsimd": [nc.gpsimd],
        "alt3": [nc.scalar, nc.sync, nc.gpsimd],
        "sg": [nc.sync, nc.gpsimd],
        "cg": [nc.scalar, nc.gpsimd],
    }
    out_engines = _engs[out_eng]
    in_engines = _engs[in_eng]
    oe_idx = 0
    ie_idx = 0

    with ExitStack() as ctx:
        tc = ctx.enter_context(tile.TileContext(nc))
        consts = ctx.enter_context(tc.tile_pool(name="consts", bufs=1))
        xpool = ctx.enter_context(tc.tile_pool(name="xpool", bufs=xbufs))
        small = ctx.enter_context(tc.tile_pool(name="small", bufs=sbufs))
        psum = ctx.enter_context(
            tc.tile_pool(name="psum", bufs=pbufs, space=bass.MemorySpace.PSUM)
        )

        cdma = nc.gpsimd.dma_start

        if fast_head:
            # hoist the ACT function-table loads out of the first-gate chain
            warm = consts.tile([P, 1], F32, name="warm")
            nc.vector.memset(warm[:], 0.0)
            nc.scalar.activation(
                out=warm[:], in_=warm[:], func=mybir.ActivationFunctionType.Sigmoid
            )
            nc.scalar.activation(
                out=warm[:], in_=warm[:], func=mybir.ActivationFunctionType.Relu
            )
            # most-urgent first: the first scan needs gamma; SWDGE gens are
            # ~1us each and serialize on the Q7, so order by consumer time
            g_sb = consts.tile([P, NCT], F32, name="g_sb")
            cdma(out=g_sb[:], in_=g_ext[:])

        w1_sb = consts.tile([P, NCT, HID], MMDT)
        cdma(out=w1_sb[:], in_=w1_ext[:].rearrange("p (ct h) -> p ct h", ct=NCT))
        w2_sb = consts.tile([HID, C], MMDT)
        cdma(out=w2_sb[:], in_=w2_ext[:])
        b1_sb = consts.tile([HID, 1], F32)
        cdma(out=b1_sb[:], in_=b1_ext[:])
        b2_sb = consts.tile([P, NCT], F32)
        cdma(out=b2_sb[:], in_=b2_ext[:])
        if not fast_head:
            g_sb = consts.tile([P, NCT], F32, name="g_sb")
        if g_direct:
            # gamma read directly via stride-0 broadcast APs in the scan
            nc.sync.dma_start(out=g_sb[:], in_=g_ext[:])

            def g_data0(ct, NCc):
                g_ap = g_sb[:, ct : ct + 1]
                return bass.AP(
                    tensor=g_ap.tensor,
                    offset=g_ap.offset,
                    ap=[list(g_ap.ap[0]), [0, NCc]],
                )
        else:
            if not fast_head:
                cdma(out=g_sb[:], in_=g_ext[:])
            # broadcast gamma along the free axis for the scan's data0 operand
            ones = consts.tile([P, NCmax], F32)
            nc.vector.memset(ones[:], 1.0)
            g_bcast = []
            for ct in range(NCT):
                gb = consts.tile([P, NCmax], F32, tag=f"gb{ct}")
                nc.vector.tensor_scalar_mul(gb[:], ones[:], g_sb[:, ct : ct + 1])
                g_bcast.append(gb)

            def g_data0(ct, NCc):
                return g_bcast[ct][:, :NCc]

        from concourse.tile_rust import add_dep_helper

        prev_rep_last_out = None
        for _r in range(reps):
            rep_in_insts = []
            last_out_inst = None
            u_prev = [None] * NCT
            prev_nc = 0
            col = 0
            x_big = None
            if resident:
                x_big = [
                    xpool.tile([P, L], F32, tag=f"xbig{ct}", name=f"xbig{ct}")
                    for ct in range(NCT)
                ]
            if hoist_in:
                assert resident
                c0 = 0
                for LC in chunks:
                    for ct in range(NCT):
                        in_engines[ie_idx % len(in_engines)].dma_start(
                            out=x_big[ct][:, c0 : c0 + LC],
                            in_=x_ext[ct * P : (ct + 1) * P, c0 : c0 + LC],
                        )
                        ie_idx += 1
                    c0 += LC
            for k, LC in enumerate(chunks):
                NCc = LC // CS
                xts = []
                uts = []
                for ct in range(NCT):
                    if resident:
                        x_t = x_big[ct][:, col : col + LC]
                    else:
                        x_t = xpool.tile([P, LC], F32, tag=f"x{ct}", name=f"x{ct}")[:]
                    if not hoist_in:
                        in_inst = in_engines[ie_idx % len(in_engines)].dma_start(
                            out=x_t,
                            in_=x_ext[ct * P : (ct + 1) * P, col : col + LC],
                        )
                        ie_idx += 1
                        if serialize and prev_rep_last_out is not None:
                            add_dep_helper(
                                in_inst.ins,
                                prev_rep_last_out.ins,
                                reason="serialize reps for single-shot timing",
                            )
                    if dma_only:
                        out_engines[oe_idx % len(out_engines)].dma_start(
                            out=out_ext[ct * P : (ct + 1) * P, col : col + LC],
                            in_=x_t,
                        )
                        oe_idx += 1
                        xts.append(x_t)
                        continue
                    xc_t = small.tile([P, NCc], F32, tag=f"xc{ct}")
                    nc.vector.tensor_reduce(
                        out=xc_t[:],
                        in_=x_t.rearrange("p (n j) -> p n j", j=CS),
                        axis=mybir.AxisListType.X,
                        op=mybir.AluOpType.add,
                    )
                    u_t = small.tile([P, NCc], MMDT, tag=f"u{ct}")
                    init = 0.0 if k == 0 else u_prev[ct][:, prev_nc - 1 : prev_nc]
                    nc.vector.tensor_tensor_scan(
                        out=u_t[:],
                        data0=g_data0(ct, NCc),
                        data1=xc_t[:],
                        initial=init,
                        op0=mybir.AluOpType.mult,
                        op1=mybir.AluOpType.add,
                    )
                    xts.append(x_t)
                    uts.append(u_t)
                if dma_only:
                    col += LC
                    continue

                # SE bottleneck: h = relu(w1s @ u + b1)
                h_ps = psum.tile([HID, NCc], F32, tag="hps")
                for ct in range(NCT):
                    nc.tensor.matmul(
                        h_ps[:],
                        w1_sb[:, ct, :],
                        uts[ct][:],
                        start=(ct == 0),
                        stop=(ct == NCT - 1),
                    )
                h_sb = small.tile([HID, NCc], MMDT, tag="h")
                nc.scalar.activation(
                    out=h_sb[:],
                    in_=h_ps[:],
                    func=mybir.ActivationFunctionType.Relu,
                    bias=b1_sb[:],
                )
                for ct in range(NCT):
                    o_ps = psum.tile([P, NCc], F32, tag="ops")
                    nc.tensor.matmul(
                        o_ps[:],
                        w2_sb[:, ct * P : (ct + 1) * P],
                        h_sb[:],
                        start=True,
                        stop=True,
                    )
                    gate_t = small.tile([P, NCc], F32, tag="gate")
                    nc.scalar.activation(
                        out=gate_t[:],
                        in_=o_ps[:],
                        func=mybir.ActivationFunctionType.Sigmoid,
                        bias=b2_sb[:, ct : ct + 1],
                    )
                    x3 = xts[ct].rearrange("p (n j) -> p n j", j=CS)
                    g_ap = gate_t[:]
                    gate_b = bass.AP(
                        tensor=g_ap.tensor,
                        offset=g_ap.offset,
                        ap=[list(g_ap.ap[0]), list(g_ap.ap[1]), [0, CS]],
                    )
                    gate_eng = nc.gpsimd if ct < gp_gate else nc.vector
                    gate_eng.tensor_tensor(
                        out=x3, in0=x3, in1=gate_b, op=mybir.AluOpType.mult
                    )
                    last_out_inst = out_engines[oe_idx % len(out_engines)].dma_start(
                        out=out_ext[ct * P : (ct + 1) * P, col : col + LC],
                        in_=xts[ct],
                    )
                    oe_idx += 1
                u_prev = uts
                prev_nc = NCc
                col += LC
            prev_rep_last_out = last_out_inst

    nc.compile()
    return nc


def host_prep(gamma, w1, b1, w2, b2, C=512, HID=64, se_bf16=True):
    """Host-side preprocessing of the shared (small) tensors."""
    import ml_dtypes

    NCT = C // P
    mmdt = ml_dtypes.bfloat16 if se_bf16 else np.float32
    gamma = np.asarray(gamma, np.float32)
    w1 = np.asarray(w1, np.float32)
    w2 = np.asarray(w2, np.float32)
    bv = (1.0 - gamma) / 16.0
    w1s = (w1 * bv[None, :]).T  # [C, HID]
    # [C, HID] -> [P, NCT*HID] with c = ct*P + p
    w1s_r = np.ascontiguousarray(
        w1s.reshape(NCT, P, HID).transpose(1, 0, 2).reshape(P, NCT * HID)
    ).astype(mmdt)
    w2t = np.ascontiguousarray(w2.T).astype(mmdt)  # [HID, C]
    b1_r = np.ascontiguousarray(np.asarray(b1, np.float32).reshape(HID, 1))
    b2_r = np.ascontiguousarray(np.asarray(b2, np.float32).reshape(NCT, P).T)
    g_r = np.ascontiguousarray(gamma.reshape(NCT, P).T)
    return w1s_r, w2t, b1_r, b2_r, g_r


DEFAULT_NL = 4
DEFAULT_CFG = dict(
    chunks=[512, 1024, 1536, 2048, 1536, 1024, 512],
    in_eng="alt",
    out_eng="alt",
    xbufs=1,
    sbufs=3,
    pbufs=3,
    se_bf16=False,
    resident=True,
)

_GRAPH_CACHE = {}


def _get_graph(reps=1):
    key = reps
    if key not in _GRAPH_CACHE:
        _GRAPH_CACHE[key] = build_graph(reps=reps, **DEFAULT_CFG)
    return _GRAPH_CACHE[key]


def make_in_maps(x, gamma, w1, b1, w2, b2):
    B, C, L = x.shape
    HID = w1.shape[0]
    w1s_r, w2t, b1_r, b2_r, g_r = host_prep(
        gamma, w1, b1, w2, b2, C=C, HID=HID, se_bf16=DEFAULT_CFG["se_bf16"]
    )
    x = np.asarray(x, np.float32)
    return [
        {
            "x": np.ascontiguousarray(x[b]),
            "w1s": w1s_r,
            "w2t": w2t,
            "b1": b1_r,
            "b2": b2_r,
            "g": g_r,
        }
        for b in range(B)
    ]


_RUNNER_CACHE = {}


def _make_runner(nc, n_cores):
    """Persistent jitted SPMD runner for `nc` across `n_cores` devices.

    Returns run(in_maps) -> list[dict] of per-core outputs.
    """
    import jax
    from jax.sharding import Mesh, PartitionSpec
    from jax.experimental.shard_map import shard_map
    from concourse import bass2jax

    bass2jax.install_neuronx_cc_hook()

    partition_name = nc.partition_id_tensor.name if nc.partition_id_tensor else None
    in_names, out_names, out_avals = [], [], []
    for alloc in nc.m.functions[0].allocations:
        if not isinstance(alloc, mybir.MemoryLocationSet):
            continue
        name = alloc.memorylocations[0].name
        if alloc.kind == "ExternalInput":
            if name != partition_name:
                in_names.append(name)
        elif alloc.kind == "ExternalOutput":
            out_names.append(name)
            out_avals.append(
                jax.core.ShapedArray(tuple(alloc.tensor_shape), mybir.dt.np(alloc.dtype))
            )
    n_params = len(in_names)
    in_names_all = in_names + out_names
    if partition_name is not None:
        in_names_all.append(partition_name)

    def _body(*args):
        operands = list(args)
        if partition_name is not None:
            operands.append(bass2jax.partition_id_tensor())
        outs = bass2jax._bass_exec_p.bind(
            *operands,
            out_avals=tuple(out_avals),
            in_names=tuple(in_names_all),
            out_names=tuple(out_names),
            lowering_input_output_aliases=(),
            sim_require_finite=True,
            sim_require_nnan=True,
            nc=nc,
        )
        return tuple(outs)

    devices = jax.devices()[:n_cores]
    mesh = Mesh(np.asarray(devices), ("core",))
    n_outs = len(out_avals)
    sharded = jax.jit(
        shard_map(
            _body,
            mesh=mesh,
            in_specs=(PartitionSpec("core"),) * (n_params + n_outs),
            out_specs=(PartitionSpec("core"),) * len(out_names),
            check_rep=False,
        ),
        keep_unused=True,
    )
    concat_zeros = [
        np.zeros((n_cores * a.shape[0], *a.shape[1:]), a.dtype) for a in out_avals
    ]

    def run(in_maps):
        per_core = [[np.asarray(m[name]) for name in in_names] for m in in_maps]
        concat_in = [
            np.concatenate([per_core[c][i] for c in range(n_cores)], axis=0)
            for i in range(n_params)
        ]
        out_arrs = sharded(*concat_in, *concat_zeros)
        return [
            {
                name: np.asarray(out_arrs[i]).reshape(
                    n_cores, *out_avals[i].shape
                )[c]
                for i, name in enumerate(out_names)
            }
            for c in range(n_cores)
        ]

    return run


def _get_runner(reps=1, n_cores=8):
    key = (reps, n_cores)
    if key not in _RUNNER_CACHE:
        _RUNNER_CACHE[key] = _make_runner(_get_graph(reps=reps), n_cores)
    return _RUNNER_CACHE[key]


def kernel(x, gamma, w1, b1, w2, b2):
    x = np.asarray(x)
    B, C, L = x.shape
    assert (B, C, L) == (8, 512, 8192), (B, C, L)
    in_maps = make_in_maps(x, gamma, w1, b1, w2, b2)
    try:
        res = _get_runner(reps=1, n_cores=B)(in_maps)
    except Exception:
        # fallback: the official (slower to dispatch, identical NEFF) path
        from concourse.bass_utils import run_bass_kernel_spmd

        res = run_bass_kernel_spmd(
            _get_graph(reps=1), in_maps, core_ids=list(range(B))
        ).results
    out = np.stack([res[b]["out"] for b in range(B)], axis=0)
    return np.ascontiguousarray(out, dtype=np.float32)


# revision 54
# speedup vs baseline: 1.0556x; 1.0383x over previous
"""Trainium2 Bass kernel for nn_CausalSE (chunked-EMA squeeze-excite gating).

Reference computation (per batch b):
    xc   = mean over chunks of 16 along L            -> [C, N]   (N = L/16)
    e_t  = g*e_{t-1} + (1-g)*xc_t   (causal EMA)     -> [C, N]
    h    = relu(w1 @ e + b1)                         -> [C/8, N]
    gate = sigmoid(w2 @ h + b2)                      -> [C, N]
    out  = repeat(gate, 16) * x                      -> [C, L]

Distribution: pure data-parallel over batch. B == 8 == n_cores, each core
processes one full batch element independently; no collectives.

Math transform used on-chip: let u_t = g*u_{t-1} + sum16(x)_t (plain scan on
pooled *sums*).  Then e = ((1-g)/16) * u, which is folded into w1 on the host
(w1s = w1 * ((1-g)/16)).  This removes a per-element rescale pass on DVE.
"""

import numpy as np
from contextlib import ExitStack

import concourse.bass as bass
import concourse.tile as tile
from concourse import bacc, mybir

F32 = mybir.dt.float32
P = 128


def build_graph(C=512, L=8192, CS=16, HID=64, NL=4, reps=1, chunks=None,
                se_bf16=True, out_eng="scalar", in_eng="sync", xbufs=2, sbufs=2, pbufs=2,
                resident=False, hoist_in=False, dma_only=False,
                gp_gate=0, serialize=False, g_direct=False, fast_head=False):
    """Build the per-core Bass graph (SPMD: every core runs this same graph).

    NL: number of equal column chunks (ignored if `chunks` given).
    chunks: explicit list of column widths (each a multiple of CS, sum == L).
    reps: repeat the whole computation (for on-device timing via slope).
    se_bf16: run the SE matmuls in bf16 (scan output downcasts for free).
    """
    NCT = C // P          # channel partition-tiles
    if chunks is None:
        chunks = [L // NL] * NL
    assert sum(chunks) == L and all(c % CS == 0 for c in chunks)
    NL = len(chunks)
    LCmax = max(chunks)
    NCmax = LCmax // CS
    MMDT = mybir.dt.bfloat16 if se_bf16 else F32

    nc = bacc.Bacc(None, target_bir_lowering=False)

    x_ext = nc.declare_dram_parameter("x", [C, L], F32, isOutput=False)
    w1_ext = nc.declare_dram_parameter("w1s", [P, NCT * HID], MMDT, isOutput=False)
    w2_ext = nc.declare_dram_parameter("w2t", [HID, C], MMDT, isOutput=False)
    b1_ext = nc.declare_dram_parameter("b1", [HID, 1], F32, isOutput=False)
    b2_ext = nc.declare_dram_parameter("b2", [P, NCT], F32, isOutput=False)
    g_ext = nc.declare_dram_parameter("g", [P, NCT], F32, isOutput=False)
    out_ext = nc.declare_dram_parameter("out", [C, L], F32, isOutput=True)

    _engs = {
        "scalar": [nc.scalar],
        "sync": [nc.sync],
        "alt": [nc.scalar, nc.sync],
        "gpsimd": [nc.gpsimd],
        "alt3": [nc.scalar, nc.sync, nc.gpsimd],
        "sg": [nc.sync, nc.gpsimd],
        "cg": [nc.scalar, nc.gpsimd],
    }
    out_engines = _engs[out_eng]
    in_engines = _engs[in_eng]
    oe_idx = 0
    ie_idx = 0

    with ExitStack() as ctx:
        tc = ctx.enter_context(tile.TileContext(nc))
        consts = ctx.enter_context(tc.tile_pool(name="consts", bufs=1))
        xpool = ctx.enter_context(tc.tile_pool(name="xpool", bufs=xbufs))
        small = ctx.enter_context(tc.tile_pool(name="small", bufs=sbufs))
        psum = ctx.enter_context(
            tc.tile_pool(name="psum", bufs=pbufs, space=bass.MemorySpace.PSUM)
        )

        cdma = nc.gpsimd.dma_start

        if fast_head:
            # hoist the ACT function-table loads out of the first-gate chain
            warm = consts.tile([P, 1], F32, name="warm")
            nc.vector.memset(warm[:], 0.0)
            nc.scalar.activation(
                out=warm[:], in_=warm[:], func=mybir.ActivationFunctionType.Sigmoid
            )
            nc.scalar.activation(
                out=warm[:], in_=warm[:], func=mybir.ActivationFunctionType.Relu
            )
            # most-urgent first: the first scan needs gamma; SWDGE gens are
            # ~1us each and serialize on the Q7, so order by consumer time
            g_sb = consts.tile([P, NCT], F32, name="g_sb")
            cdma(out=g_sb[:], in_=g_ext[:])

        w1_sb = consts.tile([P, NCT, HID], MMDT)
        cdma(out=w1_sb[:], in_=w1_ext[:].rearrange("p (ct h) -> p ct h", ct=NCT))
        w2_sb = consts.tile([HID, C], MMDT)
        cdma(out=w2_sb[:], in_=w2_ext[:])
        b1_sb = consts.tile([HID, 1], F32)
        cdma(out=b1_sb[:], in_=b1_ext[:])
        b2_sb = consts.tile([P, NCT], F32)
        cdma(out=b2_sb[:], in_=b2_ext[:])
        if not fast_head:
            g_sb = consts.tile([P, NCT], F32, name="g_sb")
        if g_direct:
            # gamma read directly via stride-0 broadcast APs in the scan
            nc.sync.dma_start(out=g_sb[:], in_=g_ext[:])

            def g_data0(ct, NCc):
                g_ap = g_sb[:, ct : ct + 1]
                return bass.AP(
                    tensor=g_ap.tensor,
                    offset=g_ap.offset,
                    ap=[list(g_ap.ap[0]), [0, NCc]],
                )
        else:
            if not fast_head:
                cdma(out=g_sb[:], in_=g_ext[:])
            # broadcast gamma along the free axis for the scan's data0 operand
            ones = consts.tile([P, NCmax], F32)
            nc.vector.memset(ones[:], 1.0)
            g_bcast = []
            for ct in range(NCT):
                gb = consts.tile([P, NCmax], F32, tag=f"gb{ct}")
                nc.vector.tensor_scalar_mul(gb[:], ones[:], g_sb[:, ct : ct + 1])
                g_bcast.append(gb)

            def g_data0(ct, NCc):
                return g_bcast[ct][:, :NCc]

        from concourse.tile_rust import add_dep_helper

        prev_rep_last_out = None
        for _r in range(reps):
            rep_in_insts = []
            last_out_inst = None
            u_prev = [None] * NCT
            prev_nc = 0
            col = 0
            x_big = None
            if resident:
                x_big = [
                    xpool.tile([P, L], F32, tag=f"xbig{ct}", name=f"xbig{ct}")
                    for ct in range(NCT)
                ]
            if hoist_in:
                assert resident
                c0 = 0
                for LC in chunks:
                    for ct in range(NCT):
                        in_engines[ie_idx % len(in_engines)].dma_start(
                            out=x_big[ct][:, c0 : c0 + LC],
                            in_=x_ext[ct * P : (ct + 1) * P, c0 : c0 + LC],
                        )
                        ie_idx += 1
                    c0 += LC
            for k, LC in enumerate(chunks):
                NCc = LC // CS
                xts = []
                uts = []
                for ct in range(NCT):
                    if resident:
                        x_t = x_big[ct][:, col : col + LC]
                    else:
                        x_t = xpool.tile([P, LC], F32, tag=f"x{ct}", name=f"x{ct}")[:]
                    if not hoist_in:
                        in_inst = in_engines[ie_idx % len(in_engines)].dma_start(
                            out=x_t,
                            in_=x_ext[ct * P : (ct + 1) * P, col : col + LC],
                        )
                        ie_idx += 1
                        if serialize and prev_rep_last_out is not None:
                            add_dep_helper(
                                in_inst.ins,
                                prev_rep_last_out.ins,
                                reason="serialize reps for single-shot timing",
                            )
                    if dma_only:
                        out_engines[oe_idx % len(out_engines)].dma_start(
                            out=out_ext[ct * P : (ct + 1) * P, col : col + LC],
                            in_=x_t,
                        )
                        oe_idx += 1
                        xts.append(x_t)
                        continue
                    xc_t = small.tile([P, NCc], F32, tag=f"xc{ct}")
                    nc.vector.tensor_reduce(
                        out=xc_t[:],
                        in_=x_t.rearrange("p (n j) -> p n j", j=CS),
                        axis=mybir.AxisListType.X,
                        op=mybir.AluOpType.add,
                    )
                    u_t = small.tile([P, NCc], MMDT, tag=f"u{ct}")
                    init = 0.0 if k == 0 else u_prev[ct][:, prev_nc - 1 : prev_nc]
                    nc.vector.tensor_tensor_scan(
                        out=u_t[:],
                        data0=g_data0(ct, NCc),
                        data1=xc_t[:],
                        initial=init,
                        op0=mybir.AluOpType.mult,
                        op1=mybir.AluOpType.add,
                    )
                    xts.append(x_t)
                    uts.append(u_t)
                if dma_only:
                    col += LC
                    continue

                # SE bottleneck: h = relu(w1s @ u + b1)
                h_ps = psum.tile([HID, NCc], F32, tag="hps")
                for ct in range(NCT):
                    nc.tensor.matmul(
                        h_ps[:],
                        w1_sb[:, ct, :],
                        uts[ct][:],
                        start=(ct == 0),
                        stop=(ct == NCT - 1),
                    )
                h_sb = small.tile([HID, NCc], MMDT, tag="h")
                nc.scalar.activation(
                    out=h_sb[:],
                    in_=h_ps[:],
                    func=mybir.ActivationFunctionType.Relu,
                    bias=b1_sb[:],
                )
                for ct in range(NCT):
                    o_ps = psum.tile([P, NCc], F32, tag="ops")
                    nc.tensor.matmul(
                        o_ps[:],
                        w2_sb[:, ct * P : (ct + 1) * P],
                        h_sb[:],
                        start=True,
                        stop=True,
                    )
                    gate_t = small.tile([P, NCc], F32, tag="gate")
                    nc.scalar.activation(
                        out=gate_t[:],
                        in_=o_ps[:],
                        func=mybir.ActivationFunctionType.Sigmoid,
                        bias=b2_sb[:, ct : ct + 1],
                    )
                    x3 = xts[ct].rearrange("p (n j) -> p n j", j=CS)
                    g_ap = gate_t[:]
                    gate_b = bass.AP(
                        tensor=g_ap.tensor,
                        offset=g_ap.offset,
                        ap=[list(g_ap.ap[0]), list(g_ap.ap[1]), [0, CS]],
                    )
                    gate_eng = nc.gpsimd if ct < gp_gate else nc.vector
                    gate_eng.tensor_tensor(
                        out=x3, in0=x3, in1=gate_b, op=mybir.AluOpType.mult
                    )
                    last_out_inst = out_engines[oe_idx % len(out_engines)].dma_start(
                        out=out_ext[ct * P : (ct + 1) * P, col : col + LC],
                        in_=xts[ct],
                    )
                    oe_idx += 1
                u_prev = uts
                prev_nc = NCc
                col += LC
            prev_rep_last_out = last_out_inst

    nc.compile()
    return nc


def host_prep(gamma, w1, b1, w2, b2, C=512, HID=64, se_bf16=True):
    """Host-side preprocessing of the shared (small) tensors."""
    import ml_dtypes

    NCT = C // P
    mmdt = ml_dtypes.bfloat16 if se_bf16 else np.float32
    gamma = np.asarray(gamma, np.float32)
    w1 = np.asarray(w1, np.float32)
    w2 = np.asarray(w2, np.float32)
    bv = (1.0 - gamma) / 16.0
    w1s = (w1 * bv[None, :]).T  # [C, HID]
    # [C, HID] -> [P, NCT*HID] with c = ct*P + p
    w1s_r = np.ascontiguousarray(
        w1s.reshape(NCT, P, HID).transpose(1, 0, 2).reshape(P, NCT * HID)
    ).astype(mmdt)
    w2t = np.ascontiguousarray(w2.T).astype(mmdt)  # [HID, C]
    b1_r = np.ascontiguousarray(np.asarray(b1, np.float32).reshape(HID, 1))
    b2_r = np.ascontiguousarray(np.asarray(b2, np.float32).reshape(NCT, P).T)
    g_r = np.ascontiguousarray(gamma.reshape(NCT, P).T)
    return w1s_r, w2t, b1_r, b2_r, g_r


DEFAULT_NL = 4
DEFAULT_CFG = dict(
    chunks=[512, 1024, 1536, 2048, 1536, 1024, 512],
    in_eng="alt",
    out_eng="alt",
    xbufs=1,
    sbufs=3,
    pbufs=3,
    se_bf16=False,
    resident=True,
)

_GRAPH_CACHE = {}


def _get_graph(reps=1):
    key = reps
    if key not in _GRAPH_CACHE:
        _GRAPH_CACHE[key] = build_graph(reps=reps, **DEFAULT_CFG)
    return _GRAPH_CACHE[key]


def make_in_maps(x, gamma, w1, b1, w2, b2):
    B, C, L = x.shape
    HID = w1.shape[0]
    w1s_r, w2t, b1_r, b2_r, g_r = host_prep(
        gamma, w1, b1, w2, b2, C=C, HID=HID, se_bf16=DEFAULT_CFG["se_bf16"]
    )
    x = np.ascontiguousarray(np.asarray(x, np.float32))
    return [
        {
            "x": x[b],  # view of the contiguous parent -> no copy downstream
            "w1s": w1s_r,
            "w2t": w2t,
            "b1": b1_r,
            "b2": b2_r,
            "g": g_r,
        }
        for b in range(B)
    ]


_RUNNER_CACHE = {}


def _make_runner(nc, n_cores):
    """Persistent jitted SPMD runner for `nc` across `n_cores` devices.

    Returns run(in_maps) -> list[dict] of per-core outputs.
    """
    import jax
    from jax.sharding import Mesh, PartitionSpec
    from jax.experimental.shard_map import shard_map
    from concourse import bass2jax

    bass2jax.install_neuronx_cc_hook()

    partition_name = nc.partition_id_tensor.name if nc.partition_id_tensor else None
    in_names, out_names, out_avals = [], [], []
    for alloc in nc.m.functions[0].allocations:
        if not isinstance(alloc, mybir.MemoryLocationSet):
            continue
        name = alloc.memorylocations[0].name
        if alloc.kind == "ExternalInput":
            if name != partition_name:
                in_names.append(name)
        elif alloc.kind == "ExternalOutput":
            out_names.append(name)
            out_avals.append(
                jax.core.ShapedArray(tuple(alloc.tensor_shape), mybir.dt.np(alloc.dtype))
            )
    n_params = len(in_names)
    in_names_all = in_names + out_names
    if partition_name is not None:
        in_names_all.append(partition_name)

    def _body(*args):
        operands = list(args)
        if partition_name is not None:
            operands.append(bass2jax.partition_id_tensor())
        outs = bass2jax._bass_exec_p.bind(
            *operands,
            out_avals=tuple(out_avals),
            in_names=tuple(in_names_all),
            out_names=tuple(out_names),
            lowering_input_output_aliases=(),
            sim_require_finite=True,
            sim_require_nnan=True,
            nc=nc,
        )
        return tuple(outs)

    devices = jax.devices()[:n_cores]
    mesh = Mesh(np.asarray(devices), ("core",))
    n_outs = len(out_avals)
    sharded = jax.jit(
        shard_map(
            _body,
            mesh=mesh,
            in_specs=(PartitionSpec("core"),) * (n_params + n_outs),
            out_specs=(PartitionSpec("core"),) * len(out_names),
            check_rep=False,
        ),
        keep_unused=True,
    )
    concat_zeros = [
        np.zeros((n_cores * a.shape[0], *a.shape[1:]), a.dtype) for a in out_avals
    ]

    def run(in_maps):
        concat_in = []
        for i, name in enumerate(in_names):
            parts = [np.asarray(m[name]) for m in in_maps]
            base = parts[0].base if parts[0].base is not None else parts[0]
            if (
                base.ndim == parts[0].ndim + 1
                and base.shape[0] == n_cores
                and base.flags.c_contiguous
                and all(
                    p.base is base
                    and p.__array_interface__["data"][0]
                    == base.__array_interface__["data"][0] + c * parts[0].nbytes
                    for c, p in enumerate(parts)
                )
            ):
                # per-core slices of one contiguous parent: reshape, no copy
                concat_in.append(
                    np.ascontiguousarray(base).reshape(
                        n_cores * parts[0].shape[0], *parts[0].shape[1:]
                    )
                )
            else:
                concat_in.append(np.concatenate(parts, axis=0))
        out_arrs = sharded(*concat_in, *concat_zeros)
        return [
            {
                name: np.asarray(out_arrs[i]).reshape(
                    n_cores, *out_avals[i].shape
                )[c]
                for i, name in enumerate(out_names)
            }
            for c in range(n_cores)
        ]

    def run_full(in_maps):
        """Like run() but returns the first output as one stacked array
        [n_cores, ...] with a single host copy."""
        concat_in = []
        for name in in_names:
            parts = [np.asarray(m[name]) for m in in_maps]
            base = parts[0].base if parts[0].base is not None else parts[0]
            if (
                base.ndim == parts[0].ndim + 1
                and base.shape[0] == n_cores
                and base.flags.c_contiguous
                and all(
                    p.base is base
                    and p.__array_interface__["data"][0]
                    == base.__array_interface__["data"][0] + c * parts[0].nbytes
                    for c, p in enumerate(parts)
                )
            ):
                concat_in.append(
                    np.ascontiguousarray(base).reshape(
                        n_cores * parts[0].shape[0], *parts[0].shape[1:]
                    )
                )
            else:
                concat_in.append(np.concatenate(parts, axis=0))
        out_arrs = sharded(*concat_in, *concat_zeros)
        return np.asarray(out_arrs[0]).reshape(n_cores, *out_avals[0].shape)

    run.run_full = run_full
    return run


def _get_runner(reps=1, n_cores=8):
    key = (reps, n_cores)
    if key not in _RUNNER_CACHE:
        _RUNNER_CACHE[key] = _make_runner(_get_graph(reps=reps), n_cores)
    return _RUNNER_CACHE[key]


def kernel(x, gamma, w1, b1, w2, b2):
    x = np.asarray(x)
    B, C, L = x.shape
    assert (B, C, L) == (8, 512, 8192), (B, C, L)
    in_maps = make_in_maps(x, gamma, w1, b1, w2, b2)
    try:
        out = _get_runner(reps=1, n_cores=B).run_full(in_maps)
        return np.ascontiguousarray(out, dtype=np.float32)
    except Exception:
        # fallback: the official (slower to dispatch, identical NEFF) path
        from concourse.bass_utils import run_bass_kernel_spmd

        res = run_bass_kernel_spmd(
            _get_graph(reps=1), in_maps, core_ids=list(range(B))
        ).results
        out = np.stack([res[b]["out"] for b in range(B)], axis=0)
        return np.ascontiguousarray(out, dtype=np.float32)
